# revision 40
# baseline (speedup 1.0000x reference)
"""Causal single-head attention (B=4, S=4096, D=1024, d_key=64) on 8 trn2 cores.

Sharding: 8 cores = 4 batches x 2 key-halves. Core (b, h) handles batch b,
ALL 4096 query rows, and the 16 alternating 128-key blocks {2j+h : j=0..15}.
Each core computes partial PV numerators and softmax denominators over its
key half; the host merges the two halves per batch:
    out = (num_0 + num_1) / (den_0 + den_1).

v3 design (bf16, query-major):
  * bf16 everywhere on the data path (fp8 measured at 1.4e-2..5e-2 l2 error
    -- input quantization noise passes straight through dot products, so
    only bf16 keeps safe margin under the 2e-2 gate).
  * Query-major sweep: for each 256-row chunk c (ascending), score strips
    of up to 4 key-blocks [128, <=4, 256] accumulate in PSUM, one exp per
    strip (ACT, bf16 out), diagonal block masked by a DVE bf16 multiply,
    then per-block PV matmuls accumulate the whole chunk's numerator +
    denominator (ones-column of v) in a single PSUM tile -> one DVE copy
    per chunk into the output staging tile.
  * Projections via natural-layout matmuls (full 128-wide PE) + PE
    transposes for q/k; the transposes are emitted one strip later than
    the natural matmuls so the in-order PE never waits on the DVE
    nat-copy. v projects natural-only (PV lhsT layout, col 64 = ones).
  * Input DMA is chunk/pair-granular (0.5MB units) ordered to track the
    (c+1)-proportional work ramp; consts ride the gpsimd SWDGE queue so
    they don't delay the first data loads on the HWDGE.
"""

import numpy as np

import concourse.mybir as mybir
import concourse.tile as tile
from concourse import bacc
from concourse.bass_utils import run_bass_kernel_spmd

B, S, D, DK = 4, 4096, 1024, 64
NCORES = 8
CH = 256  # query rows per chunk
NCH = 16  # chunks per core (all 4096 rows)
KB = 128  # key block
NKP = 8  # own key-block pairs per core (16 blocks = half of 32)
DC = D // 128  # 8 contraction chunks
SLOTS = 4  # key blocks per score strip / exp call
F32 = mybir.dt.float32
BF16 = mybir.dt.bfloat16

_prog_cache = {}
_last_in_maps = None


def _build(variant):
    assert variant == "causal"

    nc = bacc.Bacc("TRN2", target_bir_lowering=False, debug=False,
                   num_devices=NCORES)

    # bf16 data, [128, unit, DC*256]: [p, u, (dc, col)] = x[128*dc+p, 256*u+col]
    qt_d = nc.declare_dram_parameter("qt", [128, NCH, DC * CH], BF16,
                                     isOutput=False)
    kt_d = nc.declare_dram_parameter("kt", [128, NKP, DC * CH], BF16,
                                     isOutput=False)
    vt_d = nc.declare_dram_parameter("vt", [128, NKP, DC * CH], BF16,
                                     isOutput=False)
    # weights [128, dc, m] = W[m, 128*dc + p]
    wq_d = nc.declare_dram_parameter("wq", [128, DC, DK], BF16, isOutput=False)
    wk_d = nc.declare_dram_parameter("wk", [128, DC, DK], BF16, isOutput=False)
    wv_d = nc.declare_dram_parameter("wv", [128, DC, DK], BF16, isOutput=False)
    mask_d = nc.declare_dram_parameter("maskq", [KB, CH], BF16, isOutput=False)
    ident_d = nc.declare_dram_parameter("ident", [128, 128], BF16,
                                        isOutput=False)
    # numerators (rows 0..63) + denominator (row 64), bf16
    out_d = nc.declare_dram_parameter("out", [DK + 1, NCH, CH], BF16,
                                      isOutput=True)

    qt4 = qt_d.rearrange("p u (dc c) -> p u dc c", c=CH)
    kt4 = kt_d.rearrange("p u (dc c) -> p u dc c", c=CH)
    vt4 = vt_d.rearrange("p u (dc c) -> p u dc c", c=CH)

    with tile.TileContext(nc) as tc:
        with (
            tc.tile_pool(name="const", bufs=1) as const,
            tc.tile_pool(name="res", bufs=1) as res,
            tc.tile_pool(name="stage", bufs=6) as stage,
            tc.tile_pool(name="natp", bufs=2) as natp,
            tc.tile_pool(name="pwork", bufs=20) as pwork,
            # PSUM budget (8 banks): scores 2x2 + PV accum 2x1 + proj 2x1
            tc.tile_pool(name="ps_s", bufs=2, space="PSUM") as ps_s,
            tc.tile_pool(name="ps_o", bufs=2, space="PSUM") as ps_o,
            tc.tile_pool(name="ps_mm", bufs=2, space="PSUM") as ps_mm,
        ):
            # PE warm-up in the initial DMA shadow: keeps the p-state ramp
            # running so the first real projections arrive at full clock.
            warm = const.tile([128, CH], BF16, tag="warm")
            nc.vector.memset(warm[:], 0.0)
            wtab = const.tile([128, 8], BF16, tag="wtab")
            # pull the Exp table load into the initial DMA shadow
            nc.scalar.activation(wtab[:], warm[:, 0:8],
                                 mybir.ActivationFunctionType.Exp, scale=1.0)
            for _ in range(10):
                wps = ps_mm.tile([DK, CH], F32, tag="mm", name="wps")
                nc.tensor.matmul(wps[:], warm[:, 0:DK], warm[:],
                                 start=True, stop=True)

            # consts are interleaved into the SP load stream (below) at the
            # positions their first consumers need them
            wq_sb = const.tile([128, DC, DK], BF16, tag="wq")
            wk_sb = const.tile([128, DC, DK], BF16, tag="wk")
            wv_sb = const.tile([128, DC, DK], BF16, tag="wv")
            msk_sb = const.tile([KB, CH], BF16, tag="msk")
            ident_sb = const.tile([128, 128], BF16, tag="ident")
            const_map = {"wk": (wk_sb, wk_d), "wq": (wq_sb, wq_d),
                         "ident": (ident_sb, ident_d), "wv": (wv_sb, wv_d),
                         "mask": (msk_sb, mask_d)}

            # projected tiles
            qts = [res.tile([DK, CH], BF16, tag=f"qt{c}", name=f"qt{c}")
                   for c in range(NCH)]
            ktp = [res.tile([DK, CH], BF16, tag=f"kt{g}", name=f"kt{g}")
                   for g in range(NKP)]
            vgp = [res.tile([128, 2, DK + 1], BF16, tag=f"vg{g}",
                            name=f"vg{g}")
                   for g in range(NKP)]
            for g in range(NKP):
                nc.vector.memset(vgp[g][:, :, DK:DK + 1], 1.0)

            # bf16 output staging (PSUM -> SBUF -> DRAM)
            osb = res.tile([DK + 1, NCH, CH], BF16, tag="osb")

            # ---- input stream -------------------------------------------
            # chunk c needs q_c + k pairs <= c//2 for scores, v pairs
            # <= c//2 for PV (trailing slightly). The tail loads q15 before
            # q14 so chunk 15's first strips (which need only old k pairs)
            # run while q14/v7 stream in.
            for sb, d in (const_map[k] for k in
                          ("wk", "wq", "ident", "wv", "mask")):
                nc.gpsimd.dma_start(sb[:], d[:])
            # q/k front-loaded (they gate the exp ladder); v deferred --
            # the PV backlog buffers in the p tiles until each v arrives
            load_order = [
                ("k", 0), ("q", 0), ("q", 1), ("k", 1), ("q", 2), ("q", 3),
                ("k", 2), ("q", 4), ("q", 5), ("k", 3), ("q", 6), ("q", 7),
                ("k", 4), ("q", 8), ("v", 0), ("q", 9), ("k", 5), ("q", 10),
                ("v", 1), ("q", 11), ("k", 6), ("q", 12), ("v", 2),
                ("q", 13), ("k", 7), ("q", 15), ("v", 3), ("q", 14),
                ("v", 4), ("v", 5), ("v", 6), ("v", 7)]

            staged = {}
            emitted = []

            def stage_load(kind, idx):
                src = {"k": kt4, "q": qt4, "v": vt4}[kind]
                st = stage.tile([128, DC, CH], BF16, tag="stage",
                                name=f"st_{kind}{idx}")
                nc.sync.dma_start(st[:], src[:, idx])
                return st

            def ensure_loaded(upto):
                for i in range(len(emitted), upto + 1):
                    kind, idx = load_order[i]
                    staged[(kind, idx)] = stage_load(kind, idx)
                    emitted.append((kind, idx))

            def prefetch(kind, idx):
                i = load_order.index((kind, idx))
                ensure_loaded(min(i + 2, len(load_order) - 1))

            # ---- projections (two-phase for q/k) ------------------------
            projected = set()
            pending_tr = []  # deferred transpose closures

            # Early ("ladder") units project DIRECTLY to the transposed
            # layout: costs +768 PE cycles per unit (M=64 array halves) but
            # skips the nat-copy -> transpose -> copy latency chain, which
            # sits on the DMA-arrival critical path while the PE is still
            # half idle. Dense-phase units use the cheaper via-transpose.
            DIRECT_Q = 0  # q chunks < this project directly
            DIRECT_K = 0  # k pairs < this project directly

            def ensure_projected(kind, idx):
                """Phase 1: natural-layout matmuls + nat copy. For q/k the
                PE transpose (phase 2) is queued on pending_tr and emitted
                at a later strip boundary so the in-order PE is not stalled
                waiting on the DVE nat-copy."""
                if (kind, idx) in projected:
                    return
                projected.add((kind, idx))
                prefetch(kind, idx)
                st = staged.pop((kind, idx))
                if kind == "v":
                    ps = ps_mm.tile([128, 2, DK], F32, tag="mm", name="ps_v")
                    for j2 in range(2):
                        for dc in range(DC):
                            nc.tensor.matmul(
                                ps[:, j2, :],
                                st[:, dc, j2 * KB:(j2 + 1) * KB],
                                wv_sb[:, dc, :],
                                start=(dc == 0), stop=(dc == DC - 1))
                    nc.vector.tensor_copy(vgp[idx][:, :, 0:DK], ps[:])
                    return
                w_sb = wq_sb if kind == "q" else wk_sb
                dst = qts[idx] if kind == "q" else ktp[idx]
                if (kind == "q" and idx < DIRECT_Q) or \
                        (kind == "k" and idx < DIRECT_K):
                    ps = ps_mm.tile([DK, CH], F32, tag="mm", name="ps_dir")
                    for dc in range(DC):
                        nc.tensor.matmul(
                            ps[:], w_sb[:, dc, :], st[:, dc, :],
                            start=(dc == 0), stop=(dc == DC - 1))
                    nc.vector.tensor_copy(dst[:], ps[:])
                    return
                ps = ps_mm.tile([128, 2, DK], F32, tag="mm", name="ps_nat")
                for j2 in range(2):
                    for dc in range(DC):
                        nc.tensor.matmul(
                            ps[:, j2, :],
                            st[:, dc, j2 * KB:(j2 + 1) * KB],
                            w_sb[:, dc, :],
                            start=(dc == 0), stop=(dc == DC - 1))
                nat = natp.tile([128, 2, DK], BF16, tag="nat")
                nc.vector.tensor_copy(nat[:], ps[:])

                def phase2(nat=nat, dst=dst):
                    pt = ps_mm.tile([DK, CH], BF16, tag="mm", name="ps_t")
                    for j2 in range(2):
                        nc.tensor.matmul(pt[:, j2 * KB:(j2 + 1) * KB],
                                         nat[:, j2, :], ident_sb[:],
                                         start=True, stop=True,
                                         is_transpose=True)
                    nc.vector.tensor_copy(dst[:], pt[:])
                pending_tr.append(phase2)

            def flush_tr():
                while pending_tr:
                    pending_tr.pop(0)()

            # ---- attention ----------------------------------------------
            pending = []  # strips awaiting PV emission
            opses = {}

            def emit_pv(item):
                c, b0, nb, p = item["c"], item["b0"], item["nb"], item["p"]
                if c not in opses:
                    opses[c] = ps_o.tile([DK + 1, CH], F32, tag="o",
                                         name=f"o{c}")
                o_ps = opses[c][:]
                for u in range(nb):
                    blk = b0 + u
                    ensure_projected("v", blk // 2)
                    nc.tensor.matmul(
                        o_ps, vgp[blk // 2][:, blk % 2], p[:, u, :],
                        start=(blk == 0), stop=(blk == c),
                        skip_group_check=True)
                if b0 + nb - 1 == c:
                    # chunk finished: stage partials. Batched stores on the
                    # gpsimd SWDGE mid-run; per-chunk sync stores at the
                    # tail so the final store chain is short.
                    nc.vector.tensor_copy(osb[:, c, :], o_ps)
                    del opses[c]
                    if c in (3, 7, 11):
                        nc.gpsimd.dma_start(out_d[:, c - 3:c + 1, :],
                                            osb[:, c - 3:c + 1, :])
                    elif c >= 12:
                        nc.sync.dma_start(out_d[:, c:c + 1, :],
                                          osb[:, c:c + 1, :])

            PMAX = 18  # PV backlog cap (p tiles buffer it)

            def v_emitted(item):
                g = (item["b0"] + item["nb"] - 1) // 2
                return ("v", g) in emitted

            def drain(upto):
                # keep PV close behind the exps, but hold strips whose v
                # hasn't been loaded yet (up to PMAX of backlog) so a late
                # v never stalls the in-order PE mid-stream
                while len(pending) > upto:
                    if not v_emitted(pending[0]) and len(pending) < PMAX:
                        break
                    emit_pv(pending.pop(0))

            def strip(c, b0, nb):
                """Scores + exp (+ boundary mask) for blocks b0..b0+nb-1."""
                s_ps = ps_s.tile([KB, SLOTS, CH], F32, tag="s", name="s_ps")
                for u in range(nb):
                    blk = b0 + u
                    nc.tensor.matmul(
                        s_ps[:, u, :],
                        ktp[blk // 2][:, (blk % 2) * KB:(blk % 2 + 1) * KB],
                        qts[c][:], start=True, stop=True)
                p = pwork.tile([KB, SLOTS, CH], BF16, tag="p")
                nc.scalar.activation(
                    p[:, 0:nb, :], s_ps[:, 0:nb, :],
                    mybir.ActivationFunctionType.Exp, scale=0.125)
                if b0 + nb - 1 == c:
                    # causal boundary: the diagonal block is the last one
                    nc.vector.tensor_mul(p[:, nb - 1, :], p[:, nb - 1, :],
                                         msk_sb[:])
                pending.append(dict(c=c, b0=b0, nb=nb, p=p))

            # bootstrap: q0/k0 must be fully projected before chunk 0
            ensure_projected("k", 0)
            ensure_projected("q", 0)
            flush_tr()

            # processing plan: (chunk, first block, end block). Chunk 15's
            # first 12 blocks (old k pairs + q15 only) run before chunk 14
            # so the tail after the last loads is short.
            plan = [(c, 0, c + 1) for c in range(14)]
            plan += [(15, 0, 12), (14, 0, 15), (15, 12, 16)]
            for i, (c, blk0, blk1) in enumerate(plan):
                # project the next plan item's q one item ahead (phase 1
                # now, transposes flushed at the next strip boundary)
                if i + 1 < len(plan):
                    ensure_projected("q", plan[i + 1][0])
                g_next = (c + 2) // 2
                if g_next < NKP:
                    ensure_projected("k", g_next)
                for b0 in range(blk0, blk1, SLOTS):
                    nb = min(SLOTS, blk1 - b0)
                    for blk in range(b0, b0 + nb):
                        ensure_projected("k", blk // 2)
                    # scores first (unblocks ACT asap), then PV of the
                    # previous strip and deferred transposes fill the PE
                    # while the exp runs
                    strip(c, b0, nb)
                    drain(2)
                    flush_tr()
            drain(0)

    nc.compile()
    return nc


def _get_prog(variant):
    if variant not in _prog_cache:
        _prog_cache[variant] = _build(variant)
    return _prog_cache[variant]


def _mask_block(h):
    """Multiplicative boundary mask [KB, CH] for the diagonal own-block of
    every chunk of core-half h: local key row kappa (global key 256c + 128h
    + kappa) allows query column i (global row 256c + i) iff
    i >= kappa + 128h."""
    i = np.arange(CH)[None, :]
    kap = np.arange(KB)[:, None]
    return (i >= kap + 128 * h).astype(np.float32)


def _pack_data(x_t, bf16):
    """[D, S'] fp32 -> [128, S'//256, DC*256] bf16 with
    out[p, u, (dc c)] = x[128*dc + p, 256*u + c]."""
    Dd, Sp = x_t.shape
    v = x_t.reshape(DC, 128, Sp // CH, CH)  # dc, p, u, c
    v = v.transpose(1, 2, 0, 3)  # p, u, dc, c
    return np.ascontiguousarray(v.reshape(128, Sp // CH, DC * CH)).astype(bf16)


def kernel(queries, keys, values, Wq, Wk, Wv, mask):
    import ml_dtypes  # noqa: F401  registers numpy bfloat16

    bf16 = np.dtype("bfloat16")
    queries = np.asarray(queries, dtype=np.float32)
    keys = np.asarray(keys, dtype=np.float32)
    values = np.asarray(values, dtype=np.float32)
    mask_np = np.asarray(mask)

    causal = bool(np.array_equal(
        mask_np != 0, np.tril(np.ones((S, S), dtype=bool))))
    if not causal:
        raise NotImplementedError("only the causal mask is supported")

    qt_f = queries.transpose(0, 2, 1)  # [B, D, S]
    kt_blk = keys.transpose(0, 2, 1).reshape(B, D, S // KB, KB)
    vt_blk = values.transpose(0, 2, 1).reshape(B, D, S // KB, KB)

    def pack_w(W):
        # [DK, D] -> [128, DC, DK]: w[p, dc, m] = W[m, 128*dc + p]
        Wt = np.asarray(W, dtype=np.float32).T.reshape(DC, 128, DK)
        return np.ascontiguousarray(Wt.transpose(1, 0, 2)).astype(bf16)

    wq, wk, wv = pack_w(Wq), pack_w(Wk), pack_w(Wv)
    ident = np.eye(128, dtype=np.float32).astype(bf16)

    in_maps = []
    for core in range(NCORES):
        b, h = divmod(core, 2)
        kth = kt_blk[b, :, h::2, :].reshape(D, NKP * 2 * KB)
        vth = vt_blk[b, :, h::2, :].reshape(D, NKP * 2 * KB)
        m = {"qt": _pack_data(qt_f[b], bf16),
             "kt": _pack_data(kth, bf16),
             "vt": _pack_data(vth, bf16),
             "wq": wq, "wk": wk, "wv": wv, "ident": ident,
             "maskq": _mask_block(h).astype(bf16)}
        in_maps.append(m)

    global _last_in_maps
    _last_in_maps = in_maps
    nc = _get_prog("causal")
    res = run_bass_kernel_spmd(nc, in_maps, list(range(NCORES)))

    out = np.empty((B, S, DK), dtype=np.float32)
    ov = out.reshape(B, NCH, CH, DK)
    for b in range(B):
        r0 = np.asarray(res.results[2 * b]["out"], dtype=np.float32)
        r1 = np.asarray(res.results[2 * b + 1]["out"], dtype=np.float32)
        num = r0[:DK] + r1[:DK]  # [DK, NCH, CH]
        den = r0[DK:DK + 1] + r1[DK:DK + 1]  # [1, NCH, CH]
        ov[b] = (num / den).transpose(1, 2, 0)
    return out


if __name__ == "__main__":
    rng = np.random.default_rng(0)
    q = rng.standard_normal((B, S, D), dtype=np.float32)
    k = rng.standard_normal((B, S, D), dtype=np.float32)
    v = rng.standard_normal((B, S, D), dtype=np.float32)
    sc = 1.0 / np.sqrt(D)
    wq = rng.uniform(-sc, sc, (DK, D)).astype(np.float32)
    wk = rng.uniform(-sc, sc, (DK, D)).astype(np.float32)
    wv = rng.uniform(-sc, sc, (DK, D)).astype(np.float32)
    msk = np.tril(np.ones((S, S), dtype=np.int32))
    out = kernel(queries=q, keys=k, values=v, Wq=wq, Wk=wk, Wv=wv, mask=msk)
    print("out", out.shape, out.dtype, float(np.abs(out).mean()))


# revision 42
# speedup vs baseline: 1.0117x; 1.0117x over previous
"""Causal single-head attention (B=4, S=4096, D=1024, d_key=64) on 8 trn2 cores.

Sharding: 8 cores = 4 batches x 2 key-halves. Core (b, h) handles batch b,
ALL 4096 query rows, and the 16 alternating 128-key blocks {2j+h : j=0..15}.
Each core computes partial PV numerators and softmax denominators over its
key half; the host merges the two halves per batch:
    out = (num_0 + num_1) / (den_0 + den_1).

v3 design (bf16, query-major):
  * bf16 everywhere on the data path (fp8 measured at 1.4e-2..5e-2 l2 error
    -- input quantization noise passes straight through dot products, so
    only bf16 keeps safe margin under the 2e-2 gate).
  * Query-major sweep: for each 256-row chunk c (ascending), score strips
    of up to 4 key-blocks [128, <=4, 256] accumulate in PSUM, one exp per
    strip (ACT, bf16 out), diagonal block masked by a DVE bf16 multiply,
    then per-block PV matmuls accumulate the whole chunk's numerator +
    denominator (ones-column of v) in a single PSUM tile -> one DVE copy
    per chunk into the output staging tile.
  * Projections via natural-layout matmuls (full 128-wide PE) + PE
    transposes for q/k; the transposes are emitted one strip later than
    the natural matmuls so the in-order PE never waits on the DVE
    nat-copy. v projects natural-only (PV lhsT layout, col 64 = ones).
  * Input DMA is chunk/pair-granular (0.5MB units) ordered to track the
    (c+1)-proportional work ramp; consts ride the gpsimd SWDGE queue so
    they don't delay the first data loads on the HWDGE.
"""

import numpy as np

import concourse.mybir as mybir
import concourse.tile as tile
from concourse import bacc
from concourse.bass_utils import run_bass_kernel_spmd

B, S, D, DK = 4, 4096, 1024, 64
NCORES = 8
CH = 256  # query rows per chunk
NCH = 16  # chunks per core (all 4096 rows)
KB = 128  # key block
NKP = 8  # own key-block pairs per core (16 blocks = half of 32)
DC = D // 128  # 8 contraction chunks
SLOTS = 4  # key blocks per score strip / exp call
F32 = mybir.dt.float32
BF16 = mybir.dt.bfloat16

_prog_cache = {}
_last_in_maps = None


def _build(variant):
    assert variant == "causal"

    nc = bacc.Bacc("TRN2", target_bir_lowering=False, debug=False,
                   num_devices=NCORES)

    # bf16 data, [128, unit, DC*256]: [p, u, (dc, col)] = x[128*dc+p, 256*u+col]
    qt_d = nc.declare_dram_parameter("qt", [128, NCH, DC * CH], BF16,
                                     isOutput=False)
    kt_d = nc.declare_dram_parameter("kt", [128, NKP, DC * CH], BF16,
                                     isOutput=False)
    vt_d = nc.declare_dram_parameter("vt", [128, NKP, DC * CH], BF16,
                                     isOutput=False)
    # weights [128, dc, m] = W[m, 128*dc + p]
    wq_d = nc.declare_dram_parameter("wq", [128, DC, DK], BF16, isOutput=False)
    wk_d = nc.declare_dram_parameter("wk", [128, DC, DK], BF16, isOutput=False)
    wv_d = nc.declare_dram_parameter("wv", [128, DC, DK], BF16, isOutput=False)
    mask_d = nc.declare_dram_parameter("maskq", [KB, CH], BF16, isOutput=False)
    ident_d = nc.declare_dram_parameter("ident", [128, 128], BF16,
                                        isOutput=False)
    # numerators (rows 0..63) + denominator (row 64), bf16
    out_d = nc.declare_dram_parameter("out", [DK + 1, NCH, CH], BF16,
                                      isOutput=True)

    qt4 = qt_d.rearrange("p u (dc c) -> p u dc c", c=CH)
    kt4 = kt_d.rearrange("p u (dc c) -> p u dc c", c=CH)
    vt4 = vt_d.rearrange("p u (dc c) -> p u dc c", c=CH)

    with tile.TileContext(nc) as tc:
        with (
            tc.tile_pool(name="const", bufs=1) as const,
            tc.tile_pool(name="res", bufs=1) as res,
            tc.tile_pool(name="stage", bufs=6) as stage,
            tc.tile_pool(name="natp", bufs=2) as natp,
            tc.tile_pool(name="pwork", bufs=20) as pwork,
            # PSUM budget (8 banks): scores 2x2 + PV accum 2x1 + proj 2x1
            tc.tile_pool(name="ps_s", bufs=2, space="PSUM") as ps_s,
            tc.tile_pool(name="ps_o", bufs=2, space="PSUM") as ps_o,
            tc.tile_pool(name="ps_mm", bufs=2, space="PSUM") as ps_mm,
        ):
            # PE warm-up in the initial DMA shadow: keeps the p-state ramp
            # running so the first real projections arrive at full clock.
            warm = const.tile([128, CH], BF16, tag="warm")
            nc.vector.memset(warm[:], 0.0)
            wtab = const.tile([128, 8], BF16, tag="wtab")
            # pull the Exp table load into the initial DMA shadow
            nc.scalar.activation(wtab[:], warm[:, 0:8],
                                 mybir.ActivationFunctionType.Exp, scale=1.0)
            for _ in range(10):
                wps = ps_mm.tile([DK, CH], F32, tag="mm", name="wps")
                nc.tensor.matmul(wps[:], warm[:, 0:DK], warm[:],
                                 start=True, stop=True)

            # consts are interleaved into the SP load stream (below) at the
            # positions their first consumers need them
            wq_sb = const.tile([128, DC, DK], BF16, tag="wq")
            wk_sb = const.tile([128, DC, DK], BF16, tag="wk")
            wv_sb = const.tile([128, DC, DK], BF16, tag="wv")
            msk_sb = const.tile([KB, CH], BF16, tag="msk")
            ident_sb = const.tile([128, 128], BF16, tag="ident")
            const_map = {"wk": (wk_sb, wk_d), "wq": (wq_sb, wq_d),
                         "ident": (ident_sb, ident_d), "wv": (wv_sb, wv_d),
                         "mask": (msk_sb, mask_d)}

            # projected tiles
            qts = [res.tile([DK, CH], BF16, tag=f"qt{c}", name=f"qt{c}")
                   for c in range(NCH)]
            ktp = [res.tile([DK, CH], BF16, tag=f"kt{g}", name=f"kt{g}")
                   for g in range(NKP)]
            vgp = [res.tile([128, 2, DK + 1], BF16, tag=f"vg{g}",
                            name=f"vg{g}")
                   for g in range(NKP)]
            for g in range(NKP):
                nc.vector.memset(vgp[g][:, :, DK:DK + 1], 1.0)

            # bf16 output staging (PSUM -> SBUF -> DRAM)
            osb = res.tile([DK + 1, NCH, CH], BF16, tag="osb")

            # ---- input stream -------------------------------------------
            # chunk c needs q_c + k pairs <= c//2 for scores, v pairs
            # <= c//2 for PV (trailing slightly). The tail loads q15 before
            # q14 so chunk 15's first strips (which need only old k pairs)
            # run while q14/v7 stream in.
            for sb, d in (const_map[k] for k in
                          ("wk", "wq", "ident", "wv", "mask")):
                nc.gpsimd.dma_start(sb[:], d[:])
            # q/k front-loaded (they gate the exp ladder); v deferred --
            # the PV backlog buffers in the p tiles until each v arrives
            load_order = [
                ("k", 0), ("q", 0), ("q", 1), ("k", 1), ("q", 2), ("q", 3),
                ("k", 2), ("q", 4), ("q", 5), ("k", 3), ("q", 6), ("q", 7),
                ("k", 4), ("q", 8), ("v", 0), ("q", 9), ("k", 5), ("q", 10),
                ("v", 1), ("q", 11), ("k", 6), ("q", 12), ("v", 2),
                ("q", 13), ("k", 7), ("q", 15), ("v", 3), ("q", 14),
                ("v", 4), ("v", 5), ("v", 6), ("v", 7)]

            staged = {}
            emitted = []

            def stage_load(kind, idx):
                src = {"k": kt4, "q": qt4, "v": vt4}[kind]
                st = stage.tile([128, DC, CH], BF16, tag="stage",
                                name=f"st_{kind}{idx}")
                nc.sync.dma_start(st[:], src[:, idx])
                return st

            def ensure_loaded(upto):
                for i in range(len(emitted), upto + 1):
                    kind, idx = load_order[i]
                    staged[(kind, idx)] = stage_load(kind, idx)
                    emitted.append((kind, idx))

            def prefetch(kind, idx):
                i = load_order.index((kind, idx))
                ensure_loaded(min(i + 2, len(load_order) - 1))

            # ---- projections (two-phase for q/k) ------------------------
            projected = set()
            pending_tr = []  # deferred transpose closures

            # Early ("ladder") units project DIRECTLY to the transposed
            # layout: costs +768 PE cycles per unit (M=64 array halves) but
            # skips the nat-copy -> transpose -> copy latency chain, which
            # sits on the DMA-arrival critical path while the PE is still
            # half idle. Dense-phase units use the cheaper via-transpose.
            DIRECT_Q = 0  # q chunks < this project directly
            DIRECT_K = 0  # k pairs < this project directly

            def ensure_projected(kind, idx):
                """Phase 1: natural-layout matmuls + nat copy. For q/k the
                PE transpose (phase 2) is queued on pending_tr and emitted
                at a later strip boundary so the in-order PE is not stalled
                waiting on the DVE nat-copy."""
                if (kind, idx) in projected:
                    return
                projected.add((kind, idx))
                prefetch(kind, idx)
                st = staged.pop((kind, idx))
                if kind == "v":
                    ps = ps_mm.tile([128, 2, DK], F32, tag="mm", name="ps_v")
                    for j2 in range(2):
                        for dc in range(DC):
                            nc.tensor.matmul(
                                ps[:, j2, :],
                                st[:, dc, j2 * KB:(j2 + 1) * KB],
                                wv_sb[:, dc, :],
                                start=(dc == 0), stop=(dc == DC - 1))
                    nc.vector.tensor_copy(vgp[idx][:, :, 0:DK], ps[:])
                    return
                w_sb = wq_sb if kind == "q" else wk_sb
                dst = qts[idx] if kind == "q" else ktp[idx]
                if (kind == "q" and idx < DIRECT_Q) or \
                        (kind == "k" and idx < DIRECT_K):
                    ps = ps_mm.tile([DK, CH], F32, tag="mm", name="ps_dir")
                    for dc in range(DC):
                        nc.tensor.matmul(
                            ps[:], w_sb[:, dc, :], st[:, dc, :],
                            start=(dc == 0), stop=(dc == DC - 1))
                    nc.vector.tensor_copy(dst[:], ps[:])
                    return
                ps = ps_mm.tile([128, 2, DK], F32, tag="mm", name="ps_nat")
                for j2 in range(2):
                    for dc in range(DC):
                        nc.tensor.matmul(
                            ps[:, j2, :],
                            st[:, dc, j2 * KB:(j2 + 1) * KB],
                            w_sb[:, dc, :],
                            start=(dc == 0), stop=(dc == DC - 1))
                nat = natp.tile([128, 2, DK], BF16, tag="nat")
                nc.vector.tensor_copy(nat[:], ps[:])

                def phase2(nat=nat, dst=dst):
                    pt = ps_mm.tile([DK, CH], BF16, tag="mm", name="ps_t")
                    for j2 in range(2):
                        nc.tensor.matmul(pt[:, j2 * KB:(j2 + 1) * KB],
                                         nat[:, j2, :], ident_sb[:],
                                         start=True, stop=True,
                                         is_transpose=True)
                    nc.vector.tensor_copy(dst[:], pt[:])
                pending_tr.append(phase2)

            def flush_tr():
                while pending_tr:
                    pending_tr.pop(0)()

            # ---- attention ----------------------------------------------
            pending = []  # strips awaiting PV emission
            opses = {}

            def emit_pv(item):
                c, b0, nb, p = item["c"], item["b0"], item["nb"], item["p"]
                if c not in opses:
                    opses[c] = ps_o.tile([DK + 1, CH], F32, tag="o",
                                         name=f"o{c}")
                o_ps = opses[c][:]
                for u in range(nb):
                    blk = b0 + u
                    ensure_projected("v", blk // 2)
                    nc.tensor.matmul(
                        o_ps, vgp[blk // 2][:, blk % 2], p[:, u, :],
                        start=(blk == 0), stop=(blk == c),
                        skip_group_check=True)
                if b0 + nb - 1 == c:
                    # chunk finished: stage partials. Batched stores on the
                    # gpsimd SWDGE mid-run; per-chunk sync stores at the
                    # tail so the final store chain is short.
                    nc.vector.tensor_copy(osb[:, c, :], o_ps)
                    del opses[c]
                    if c in (3, 7, 11):
                        nc.gpsimd.dma_start(out_d[:, c - 3:c + 1, :],
                                            osb[:, c - 3:c + 1, :])
                    elif c >= 12:
                        nc.sync.dma_start(out_d[:, c:c + 1, :],
                                          osb[:, c:c + 1, :])

            PMAX = 18  # PV backlog cap (p tiles buffer it)

            def v_emitted(item):
                g = (item["b0"] + item["nb"] - 1) // 2
                return ("v", g) in emitted

            def drain(upto, max_pops=None):
                # keep PV close behind the exps, but hold strips whose v
                # hasn't been loaded yet (up to PMAX of backlog) so a late
                # v never stalls the in-order PE mid-stream; catch-up is
                # rate-limited so a PV burst never displaces scores
                pops = 0
                while len(pending) > upto:
                    if not v_emitted(pending[0]) and len(pending) < PMAX:
                        break
                    if max_pops is not None and pops >= max_pops:
                        break
                    emit_pv(pending.pop(0))
                    pops += 1

            def strip(c, b0, nb):
                """Scores + exp (+ boundary mask) for blocks b0..b0+nb-1."""
                s_ps = ps_s.tile([KB, SLOTS, CH], F32, tag="s", name="s_ps")
                for u in range(nb):
                    blk = b0 + u
                    nc.tensor.matmul(
                        s_ps[:, u, :],
                        ktp[blk // 2][:, (blk % 2) * KB:(blk % 2 + 1) * KB],
                        qts[c][:], start=True, stop=True)
                p = pwork.tile([KB, SLOTS, CH], BF16, tag="p")
                nc.scalar.activation(
                    p[:, 0:nb, :], s_ps[:, 0:nb, :],
                    mybir.ActivationFunctionType.Exp, scale=0.125)
                if b0 + nb - 1 == c:
                    # causal boundary: the diagonal block is the last one
                    nc.vector.tensor_mul(p[:, nb - 1, :], p[:, nb - 1, :],
                                         msk_sb[:])
                pending.append(dict(c=c, b0=b0, nb=nb, p=p))

            # bootstrap: q0/k0 must be fully projected before chunk 0
            ensure_projected("k", 0)
            ensure_projected("q", 0)
            flush_tr()

            # processing plan: (chunk, first block, end block). Chunk 15's
            # first 12 blocks (old k pairs + q15 only) run before chunk 14
            # so the tail after the last loads is short.
            plan = [(c, 0, c + 1) for c in range(14)]
            plan += [(15, 0, 12), (14, 0, 15), (15, 12, 16)]
            for i, (c, blk0, blk1) in enumerate(plan):
                # project the next plan item's q one item ahead (phase 1
                # now, transposes flushed at the next strip boundary)
                if i + 1 < len(plan):
                    ensure_projected("q", plan[i + 1][0])
                g_next = (c + 2) // 2
                if g_next < NKP:
                    ensure_projected("k", g_next)
                for b0 in range(blk0, blk1, SLOTS):
                    nb = min(SLOTS, blk1 - b0)
                    for blk in range(b0, b0 + nb):
                        ensure_projected("k", blk // 2)
                    # scores first (unblocks ACT asap), then PV of the
                    # previous strip and deferred transposes fill the PE
                    # while the exp runs
                    strip(c, b0, nb)
                    drain(2, max_pops=3)
                    flush_tr()
            drain(0)

    nc.compile()
    return nc


def _get_prog(variant):
    if variant not in _prog_cache:
        _prog_cache[variant] = _build(variant)
    return _prog_cache[variant]


def _mask_block(h):
    """Multiplicative boundary mask [KB, CH] for the diagonal own-block of
    every chunk of core-half h: local key row kappa (global key 256c + 128h
    + kappa) allows query column i (global row 256c + i) iff
    i >= kappa + 128h."""
    i = np.arange(CH)[None, :]
    kap = np.arange(KB)[:, None]
    return (i >= kap + 128 * h).astype(np.float32)


def _pack_data(x_t, bf16):
    """[D, S'] fp32 -> [128, S'//256, DC*256] bf16 with
    out[p, u, (dc c)] = x[128*dc + p, 256*u + c]."""
    Dd, Sp = x_t.shape
    v = x_t.reshape(DC, 128, Sp // CH, CH)  # dc, p, u, c
    v = v.transpose(1, 2, 0, 3)  # p, u, dc, c
    return np.ascontiguousarray(v.reshape(128, Sp // CH, DC * CH)).astype(bf16)


def kernel(queries, keys, values, Wq, Wk, Wv, mask):
    import ml_dtypes  # noqa: F401  registers numpy bfloat16

    bf16 = np.dtype("bfloat16")
    queries = np.asarray(queries, dtype=np.float32)
    keys = np.asarray(keys, dtype=np.float32)
    values = np.asarray(values, dtype=np.float32)
    mask_np = np.asarray(mask)

    causal = bool(np.array_equal(
        mask_np != 0, np.tril(np.ones((S, S), dtype=bool))))
    if not causal:
        raise NotImplementedError("only the causal mask is supported")

    qt_f = queries.transpose(0, 2, 1)  # [B, D, S]
    kt_blk = keys.transpose(0, 2, 1).reshape(B, D, S // KB, KB)
    vt_blk = values.transpose(0, 2, 1).reshape(B, D, S // KB, KB)

    def pack_w(W):
        # [DK, D] -> [128, DC, DK]: w[p, dc, m] = W[m, 128*dc + p]
        Wt = np.asarray(W, dtype=np.float32).T.reshape(DC, 128, DK)
        return np.ascontiguousarray(Wt.transpose(1, 0, 2)).astype(bf16)

    wq, wk, wv = pack_w(Wq), pack_w(Wk), pack_w(Wv)
    ident = np.eye(128, dtype=np.float32).astype(bf16)

    in_maps = []
    for core in range(NCORES):
        b, h = divmod(core, 2)
        kth = kt_blk[b, :, h::2, :].reshape(D, NKP * 2 * KB)
        vth = vt_blk[b, :, h::2, :].reshape(D, NKP * 2 * KB)
        m = {"qt": _pack_data(qt_f[b], bf16),
             "kt": _pack_data(kth, bf16),
             "vt": _pack_data(vth, bf16),
             "wq": wq, "wk": wk, "wv": wv, "ident": ident,
             "maskq": _mask_block(h).astype(bf16)}
        in_maps.append(m)

    global _last_in_maps
    _last_in_maps = in_maps
    nc = _get_prog("causal")
    res = run_bass_kernel_spmd(nc, in_maps, list(range(NCORES)))

    out = np.empty((B, S, DK), dtype=np.float32)
    ov = out.reshape(B, NCH, CH, DK)
    for b in range(B):
        r0 = np.asarray(res.results[2 * b]["out"], dtype=np.float32)
        r1 = np.asarray(res.results[2 * b + 1]["out"], dtype=np.float32)
        num = r0[:DK] + r1[:DK]  # [DK, NCH, CH]
        den = r0[DK:DK + 1] + r1[DK:DK + 1]  # [1, NCH, CH]
        ov[b] = (num / den).transpose(1, 2, 0)
    return out


if __name__ == "__main__":
    rng = np.random.default_rng(0)
    q = rng.standard_normal((B, S, D), dtype=np.float32)
    k = rng.standard_normal((B, S, D), dtype=np.float32)
    v = rng.standard_normal((B, S, D), dtype=np.float32)
    sc = 1.0 / np.sqrt(D)
    wq = rng.uniform(-sc, sc, (DK, D)).astype(np.float32)
    wk = rng.uniform(-sc, sc, (DK, D)).astype(np.float32)
    wv = rng.uniform(-sc, sc, (DK, D)).astype(np.float32)
    msk = np.tril(np.ones((S, S), dtype=np.int32))
    out = kernel(queries=q, keys=k, values=v, Wq=wq, Wk=wk, Wv=wv, mask=msk)
    print("out", out.shape, out.dtype, float(np.abs(out).mean()))


# revision 45
# speedup vs baseline: 1.0558x; 1.0436x over previous
"""Causal single-head attention (B=4, S=4096, D=1024, d_key=64) on 8 trn2 cores.

Sharding: 8 cores = 4 batches x 2 key-halves. Core (b, h) handles batch b,
ALL 4096 query rows, and the 16 alternating 128-key blocks {2j+h : j=0..15}.
Each core computes partial PV numerators and softmax denominators over its
key half; the host merges the two halves per batch:
    out = (num_0 + num_1) / (den_0 + den_1).

v3 design (bf16, query-major):
  * bf16 everywhere on the data path (fp8 measured at 1.4e-2..5e-2 l2 error
    -- input quantization noise passes straight through dot products, so
    only bf16 keeps safe margin under the 2e-2 gate).
  * Query-major sweep: for each 256-row chunk c (ascending), score strips
    of up to 4 key-blocks [128, <=4, 256] accumulate in PSUM, one exp per
    strip (ACT, bf16 out), diagonal block masked by a DVE bf16 multiply,
    then per-block PV matmuls accumulate the whole chunk's numerator +
    denominator (ones-column of v) in a single PSUM tile -> one DVE copy
    per chunk into the output staging tile.
  * Projections via natural-layout matmuls (full 128-wide PE) + PE
    transposes for q/k; the transposes are emitted one strip later than
    the natural matmuls so the in-order PE never waits on the DVE
    nat-copy. v projects natural-only (PV lhsT layout, col 64 = ones).
  * Input DMA is chunk/pair-granular (0.5MB units) ordered to track the
    (c+1)-proportional work ramp; consts ride the gpsimd SWDGE queue so
    they don't delay the first data loads on the HWDGE.
"""

import numpy as np

import concourse.mybir as mybir
import concourse.tile as tile
from concourse import bacc
from concourse.bass_utils import run_bass_kernel_spmd

B, S, D, DK = 4, 4096, 1024, 64
NCORES = 8
CH = 256  # query rows per chunk
NCH = 16  # chunks per core (all 4096 rows)
KB = 128  # key block
NKP = 8  # own key-block pairs per core (16 blocks = half of 32)
DC = D // 128  # 8 contraction chunks
SLOTS = 4  # key blocks per score strip / exp call
F32 = mybir.dt.float32
BF16 = mybir.dt.bfloat16

_prog_cache = {}
_last_in_maps = None


def _build(variant):
    assert variant == "causal"

    nc = bacc.Bacc("TRN2", target_bir_lowering=False, debug=False,
                   num_devices=NCORES)

    # bf16 data, [128, unit, DC*256]: [p, u, (dc, col)] = x[128*dc+p, 256*u+col]
    qt_d = nc.declare_dram_parameter("qt", [128, NCH, DC * CH], BF16,
                                     isOutput=False)
    kt_d = nc.declare_dram_parameter("kt", [128, NKP, DC * CH], BF16,
                                     isOutput=False)
    vt_d = nc.declare_dram_parameter("vt", [128, NKP, DC * CH], BF16,
                                     isOutput=False)
    # weights [128, dc, m] = W[m, 128*dc + p]
    wq_d = nc.declare_dram_parameter("wq", [128, DC, DK], BF16, isOutput=False)
    wk_d = nc.declare_dram_parameter("wk", [128, DC, DK], BF16, isOutput=False)
    wv_d = nc.declare_dram_parameter("wv", [128, DC, DK], BF16, isOutput=False)
    mask_d = nc.declare_dram_parameter("maskq", [KB, CH], BF16, isOutput=False)
    ident_d = nc.declare_dram_parameter("ident", [128, 128], BF16,
                                        isOutput=False)
    # numerators (rows 0..63) + denominator (row 64), bf16
    out_d = nc.declare_dram_parameter("out", [DK + 1, NCH, CH], BF16,
                                      isOutput=True)

    qt4 = qt_d.rearrange("p u (dc c) -> p u dc c", c=CH)
    kt4 = kt_d.rearrange("p u (dc c) -> p u dc c", c=CH)
    vt4 = vt_d.rearrange("p u (dc c) -> p u dc c", c=CH)

    with tile.TileContext(nc) as tc:
        with (
            tc.tile_pool(name="const", bufs=1) as const,
            tc.tile_pool(name="res", bufs=1) as res,
            tc.tile_pool(name="stage", bufs=6) as stage,
            tc.tile_pool(name="natp", bufs=2) as natp,
            tc.tile_pool(name="pwork", bufs=5) as pwork,
            # PSUM budget (8 banks): scores 2x2 + PV accum 2x1 + proj 2x1
            tc.tile_pool(name="ps_s", bufs=2, space="PSUM") as ps_s,
            tc.tile_pool(name="ps_o", bufs=2, space="PSUM") as ps_o,
            tc.tile_pool(name="ps_mm", bufs=2, space="PSUM") as ps_mm,
        ):
            # PE warm-up in the initial DMA shadow: keeps the p-state ramp
            # running so the first real projections arrive at full clock.
            warm = const.tile([128, CH], BF16, tag="warm")
            nc.vector.memset(warm[:], 0.0)
            wtab = const.tile([128, 8], BF16, tag="wtab")
            # pull the Exp table load into the initial DMA shadow
            nc.scalar.activation(wtab[:], warm[:, 0:8],
                                 mybir.ActivationFunctionType.Exp, scale=1.0)
            for _ in range(10):
                wps = ps_mm.tile([DK, CH], F32, tag="mm", name="wps")
                nc.tensor.matmul(wps[:], warm[:, 0:DK], warm[:],
                                 start=True, stop=True)

            # consts are interleaved into the SP load stream (below) at the
            # positions their first consumers need them
            wq_sb = const.tile([128, DC, DK], BF16, tag="wq")
            wk_sb = const.tile([128, DC, DK], BF16, tag="wk")
            wv_sb = const.tile([128, DC, DK], BF16, tag="wv")
            msk_sb = const.tile([KB, CH], BF16, tag="msk")
            ident_sb = const.tile([128, 128], BF16, tag="ident")
            const_map = {"wk": (wk_sb, wk_d), "wq": (wq_sb, wq_d),
                         "ident": (ident_sb, ident_d), "wv": (wv_sb, wv_d),
                         "mask": (msk_sb, mask_d)}

            # projected tiles
            qts = [res.tile([DK, CH], BF16, tag=f"qt{c}", name=f"qt{c}")
                   for c in range(NCH)]
            ktp = [res.tile([DK, CH], BF16, tag=f"kt{g}", name=f"kt{g}")
                   for g in range(NKP)]
            vgp = [res.tile([128, 2, DK + 1], BF16, tag=f"vg{g}",
                            name=f"vg{g}")
                   for g in range(NKP)]
            for g in range(NKP):
                nc.vector.memset(vgp[g][:, :, DK:DK + 1], 1.0)

            # bf16 output staging (PSUM -> SBUF -> DRAM)
            osb = res.tile([DK + 1, NCH, CH], BF16, tag="osb")

            # ---- input stream -------------------------------------------
            # chunk c needs q_c + k pairs <= c//2 for scores, v pairs
            # <= c//2 for PV (trailing slightly). The tail loads q15 before
            # q14 so chunk 15's first strips (which need only old k pairs)
            # run while q14/v7 stream in.
            for sb, d in (const_map[k] for k in
                          ("wk", "wq", "ident", "wv", "mask")):
                nc.gpsimd.dma_start(sb[:], d[:])
            load_order = []
            for g in range(NKP - 1):
                load_order += [("k", g), ("q", 2 * g), ("q", 2 * g + 1),
                               ("v", g)]
            load_order += [("k", 7), ("q", 15), ("q", 14), ("v", 7)]

            staged = {}
            emitted = []

            def stage_load(kind, idx):
                src = {"k": kt4, "q": qt4, "v": vt4}[kind]
                st = stage.tile([128, DC, CH], BF16, tag="stage",
                                name=f"st_{kind}{idx}")
                nc.sync.dma_start(st[:], src[:, idx])
                return st

            def ensure_loaded(upto):
                for i in range(len(emitted), upto + 1):
                    kind, idx = load_order[i]
                    staged[(kind, idx)] = stage_load(kind, idx)
                    emitted.append((kind, idx))

            def prefetch(kind, idx):
                i = load_order.index((kind, idx))
                ensure_loaded(min(i + 2, len(load_order) - 1))

            # ---- projections (two-phase for q/k) ------------------------
            projected = set()
            pending_tr = []  # deferred transpose closures

            # Early ("ladder") units project DIRECTLY to the transposed
            # layout: costs +768 PE cycles per unit (M=64 array halves) but
            # skips the nat-copy -> transpose -> copy latency chain, which
            # sits on the DMA-arrival critical path while the PE is still
            # half idle. Dense-phase units use the cheaper via-transpose.
            DIRECT_Q = 0  # q chunks < this project directly
            DIRECT_K = 0  # k pairs < this project directly

            def ensure_projected(kind, idx):
                """Phase 1: natural-layout matmuls + nat copy. For q/k the
                PE transpose (phase 2) is queued on pending_tr and emitted
                at a later strip boundary so the in-order PE is not stalled
                waiting on the DVE nat-copy."""
                if (kind, idx) in projected:
                    return
                projected.add((kind, idx))
                prefetch(kind, idx)
                st = staged.pop((kind, idx))
                if kind == "v":
                    ps = ps_mm.tile([128, 2, DK], F32, tag="mm", name="ps_v")
                    for j2 in range(2):
                        for dc in range(DC):
                            nc.tensor.matmul(
                                ps[:, j2, :],
                                st[:, dc, j2 * KB:(j2 + 1) * KB],
                                wv_sb[:, dc, :],
                                start=(dc == 0), stop=(dc == DC - 1))
                    nc.vector.tensor_copy(vgp[idx][:, :, 0:DK], ps[:])
                    return
                w_sb = wq_sb if kind == "q" else wk_sb
                dst = qts[idx] if kind == "q" else ktp[idx]
                if (kind == "q" and idx < DIRECT_Q) or \
                        (kind == "k" and idx < DIRECT_K):
                    ps = ps_mm.tile([DK, CH], F32, tag="mm", name="ps_dir")
                    for dc in range(DC):
                        nc.tensor.matmul(
                            ps[:], w_sb[:, dc, :], st[:, dc, :],
                            start=(dc == 0), stop=(dc == DC - 1))
                    nc.vector.tensor_copy(dst[:], ps[:])
                    return
                ps = ps_mm.tile([128, 2, DK], F32, tag="mm", name="ps_nat")
                for j2 in range(2):
                    for dc in range(DC):
                        nc.tensor.matmul(
                            ps[:, j2, :],
                            st[:, dc, j2 * KB:(j2 + 1) * KB],
                            w_sb[:, dc, :],
                            start=(dc == 0), stop=(dc == DC - 1))
                nat = natp.tile([128, 2, DK], BF16, tag="nat")
                nc.vector.tensor_copy(nat[:], ps[:])

                def phase2(nat=nat, dst=dst):
                    pt = ps_mm.tile([DK, CH], BF16, tag="mm", name="ps_t")
                    for j2 in range(2):
                        nc.tensor.matmul(pt[:, j2 * KB:(j2 + 1) * KB],
                                         nat[:, j2, :], ident_sb[:],
                                         start=True, stop=True,
                                         is_transpose=True)
                    nc.vector.tensor_copy(dst[:], pt[:])
                pending_tr.append(phase2)

            def flush_tr():
                while pending_tr:
                    pending_tr.pop(0)()

            # ---- attention ----------------------------------------------
            pending = []  # strips awaiting PV emission
            opses = {}

            def emit_pv(item):
                c, b0, nb, p = item["c"], item["b0"], item["nb"], item["p"]
                if c not in opses:
                    opses[c] = ps_o.tile([DK + 1, CH], F32, tag="o",
                                         name=f"o{c}")
                o_ps = opses[c][:]
                for u in range(nb):
                    blk = b0 + u
                    ensure_projected("v", blk // 2)
                    nc.tensor.matmul(
                        o_ps, vgp[blk // 2][:, blk % 2], p[:, u, :],
                        start=(blk == 0), stop=(blk == c),
                        skip_group_check=True)
                if b0 + nb - 1 == c:
                    # chunk finished: stage partials. Batched stores on the
                    # gpsimd SWDGE mid-run; per-chunk sync stores at the
                    # tail so the final store chain is short.
                    nc.vector.tensor_copy(osb[:, c, :], o_ps)
                    del opses[c]
                    if c in (3, 7, 11):
                        nc.gpsimd.dma_start(out_d[:, c - 3:c + 1, :],
                                            osb[:, c - 3:c + 1, :])
                    elif c >= 12:
                        nc.sync.dma_start(out_d[:, c:c + 1, :],
                                          osb[:, c:c + 1, :])

            PMAX = 18  # PV backlog cap (p tiles buffer it)

            def v_emitted(item):
                g = (item["b0"] + item["nb"] - 1) // 2
                return ("v", g) in emitted

            def drain(upto, max_pops=None):
                # keep PV close behind the exps, but hold strips whose v
                # hasn't been loaded yet (up to PMAX of backlog) so a late
                # v never stalls the in-order PE mid-stream; catch-up is
                # rate-limited so a PV burst never displaces scores
                pops = 0
                while len(pending) > upto:
                    if not v_emitted(pending[0]) and len(pending) < PMAX:
                        break
                    if max_pops is not None and pops >= max_pops:
                        break
                    emit_pv(pending.pop(0))
                    pops += 1

            def strip(c, b0, nb):
                """Scores + exp (+ boundary mask) for blocks b0..b0+nb-1."""
                s_ps = ps_s.tile([KB, SLOTS, CH], F32, tag="s", name="s_ps")
                for u in range(nb):
                    blk = b0 + u
                    nc.tensor.matmul(
                        s_ps[:, u, :],
                        ktp[blk // 2][:, (blk % 2) * KB:(blk % 2 + 1) * KB],
                        qts[c][:], start=True, stop=True)
                p = pwork.tile([KB, SLOTS, CH], BF16, tag="p")
                nc.scalar.activation(
                    p[:, 0:nb, :], s_ps[:, 0:nb, :],
                    mybir.ActivationFunctionType.Exp, scale=0.125)
                if b0 + nb - 1 == c:
                    # causal boundary: the diagonal block is the last one
                    nc.vector.tensor_mul(p[:, nb - 1, :], p[:, nb - 1, :],
                                         msk_sb[:])
                pending.append(dict(c=c, b0=b0, nb=nb, p=p))

            # bootstrap: q0/k0 must be fully projected before chunk 0
            ensure_projected("k", 0)
            ensure_projected("q", 0)
            flush_tr()

            # processing plan: (chunk, first block, end block). Chunk 15's
            # first 12 blocks (old k pairs + q15 only) run before chunk 14
            # so the tail after the last loads is short.
            plan = [(c, 0, c + 1) for c in range(14)]
            plan += [(15, 0, 12), (14, 0, 15), (15, 12, 16)]
            for i, (c, blk0, blk1) in enumerate(plan):
                # project the next plan item's q one item ahead (phase 1
                # now, transposes flushed at the next strip boundary)
                if i + 1 < len(plan):
                    ensure_projected("q", plan[i + 1][0])
                g_next = (c + 2) // 2
                if g_next < NKP:
                    ensure_projected("k", g_next)
                for b0 in range(blk0, blk1, SLOTS):
                    nb = min(SLOTS, blk1 - b0)
                    for blk in range(b0, b0 + nb):
                        ensure_projected("k", blk // 2)
                    # scores first (unblocks ACT asap), then PV of the
                    # previous strip and deferred transposes fill the PE
                    # while the exp runs
                    strip(c, b0, nb)
                    drain(2)
                    flush_tr()
            drain(0)

    nc.compile()
    return nc


def _get_prog(variant):
    if variant not in _prog_cache:
        _prog_cache[variant] = _build(variant)
    return _prog_cache[variant]


def _mask_block(h):
    """Multiplicative boundary mask [KB, CH] for the diagonal own-block of
    every chunk of core-half h: local key row kappa (global key 256c + 128h
    + kappa) allows query column i (global row 256c + i) iff
    i >= kappa + 128h."""
    i = np.arange(CH)[None, :]
    kap = np.arange(KB)[:, None]
    return (i >= kap + 128 * h).astype(np.float32)


def _pack_data(x_t, bf16):
    """[D, S'] fp32 -> [128, S'//256, DC*256] bf16 with
    out[p, u, (dc c)] = x[128*dc + p, 256*u + c]."""
    Dd, Sp = x_t.shape
    v = x_t.reshape(DC, 128, Sp // CH, CH)  # dc, p, u, c
    v = v.transpose(1, 2, 0, 3)  # p, u, dc, c
    return np.ascontiguousarray(v.reshape(128, Sp // CH, DC * CH)).astype(bf16)


def kernel(queries, keys, values, Wq, Wk, Wv, mask):
    import ml_dtypes  # noqa: F401  registers numpy bfloat16

    bf16 = np.dtype("bfloat16")
    queries = np.asarray(queries, dtype=np.float32)
    keys = np.asarray(keys, dtype=np.float32)
    values = np.asarray(values, dtype=np.float32)
    mask_np = np.asarray(mask)

    causal = bool(np.array_equal(
        mask_np != 0, np.tril(np.ones((S, S), dtype=bool))))
    if not causal:
        raise NotImplementedError("only the causal mask is supported")

    qt_f = queries.transpose(0, 2, 1)  # [B, D, S]
    kt_blk = keys.transpose(0, 2, 1).reshape(B, D, S // KB, KB)
    vt_blk = values.transpose(0, 2, 1).reshape(B, D, S // KB, KB)

    def pack_w(W):
        # [DK, D] -> [128, DC, DK]: w[p, dc, m] = W[m, 128*dc + p]
        Wt = np.asarray(W, dtype=np.float32).T.reshape(DC, 128, DK)
        return np.ascontiguousarray(Wt.transpose(1, 0, 2)).astype(bf16)

    wq, wk, wv = pack_w(Wq), pack_w(Wk), pack_w(Wv)
    ident = np.eye(128, dtype=np.float32).astype(bf16)

    in_maps = []
    for core in range(NCORES):
        b, h = divmod(core, 2)
        kth = kt_blk[b, :, h::2, :].reshape(D, NKP * 2 * KB)
        vth = vt_blk[b, :, h::2, :].reshape(D, NKP * 2 * KB)
        m = {"qt": _pack_data(qt_f[b], bf16),
             "kt": _pack_data(kth, bf16),
             "vt": _pack_data(vth, bf16),
             "wq": wq, "wk": wk, "wv": wv, "ident": ident,
             "maskq": _mask_block(h).astype(bf16)}
        in_maps.append(m)

    global _last_in_maps
    _last_in_maps = in_maps
    nc = _get_prog("causal")
    res = run_bass_kernel_spmd(nc, in_maps, list(range(NCORES)))

    out = np.empty((B, S, DK), dtype=np.float32)
    ov = out.reshape(B, NCH, CH, DK)
    for b in range(B):
        r0 = np.asarray(res.results[2 * b]["out"], dtype=np.float32)
        r1 = np.asarray(res.results[2 * b + 1]["out"], dtype=np.float32)
        num = r0[:DK] + r1[:DK]  # [DK, NCH, CH]
        den = r0[DK:DK + 1] + r1[DK:DK + 1]  # [1, NCH, CH]
        ov[b] = (num / den).transpose(1, 2, 0)
    return out


if __name__ == "__main__":
    rng = np.random.default_rng(0)
    q = rng.standard_normal((B, S, D), dtype=np.float32)
    k = rng.standard_normal((B, S, D), dtype=np.float32)
    v = rng.standard_normal((B, S, D), dtype=np.float32)
    sc = 1.0 / np.sqrt(D)
    wq = rng.uniform(-sc, sc, (DK, D)).astype(np.float32)
    wk = rng.uniform(-sc, sc, (DK, D)).astype(np.float32)
    wv = rng.uniform(-sc, sc, (DK, D)).astype(np.float32)
    msk = np.tril(np.ones((S, S), dtype=np.int32))
    out = kernel(queries=q, keys=k, values=v, Wq=wq, Wk=wk, Wv=wv, mask=msk)
    print("out", out.shape, out.dtype, float(np.abs(out).mean()))
